# Initial kernel scaffold
#
"""MoE gate (softmax + bias-adjusted top-8 routing) Trainium2 Bass kernel.

Full inputs in, full outputs out. Token dim (B*S = 32768) is sharded 8 ways
across NeuronCores; the tiny gate weight [E,H] and expert biases [E] are
replicated. Each core computes logits = x @ W^T, fp32 softmax, top-8 by
bias-adjusted score, and normalized top-8 weights entirely on device.

Per-core layout choices:
  - x shard is shipped host-transposed as x^T [H, T_core] so the contraction
    dim (H) lands on SBUF partitions with fully contiguous DMA loads.
  - logits [128t, 64e] accumulate in PSUM over 16 H-chunks with the x^T
    chunk as the stationary PE operand and W^T streaming (plain fp32 for
    exact-precision routing decisions).
  - top-8 via DVE max8/max_index; bias[idx] gather via one-hot arithmetic.
"""

import os
import sys
from contextlib import ExitStack

import numpy as np

sys.path.insert(0, "/opt/trn_rl_repo")

import concourse.bacc as bacc
import concourse.bass as bass
import concourse.mybir as mybir
import concourse.tile as tile

B, S, H, E, K = 8, 4096, 2048, 64, 8
N_CORES = 8
T = B * S
T_CORE = T // N_CORES  # 4096 tokens per core
TG = 512               # tokens per group
NSUB = TG // 128       # 128-token subtiles per group
KH = H // 128          # contraction chunks

f32 = mybir.dt.float32
i32 = mybir.dt.int32
u16 = mybir.dt.uint16
Alu = mybir.AluOpType
Act = mybir.ActivationFunctionType
Ax = mybir.AxisListType


def build_nc(t_core=T_CORE, repeat=1):
    G = t_core // TG
    nc = bacc.Bacc("TRN2", target_bir_lowering=False, debug=False,
                   enable_asserts=False)
    xt = nc.dram_tensor("xt", [H, t_core], f32, kind="ExternalInput").ap()
    wt = nc.dram_tensor("wt", [H, E], f32, kind="ExternalInput").ap()
    eb = nc.dram_tensor("eb", [E], f32, kind="ExternalInput").ap()
    # Outputs in [128p, G, NSUB, K] layout (token = (g*NSUB+j)*128 + p) so the
    # store DMA is fully contiguous; host reorders the tiny result.
    idx_out = nc.dram_tensor("idx_out", [128, G, NSUB, K], i32,
                             kind="ExternalOutput").ap()
    w_out = nc.dram_tensor("w_out", [128, G, NSUB, K], f32,
                           kind="ExternalOutput").ap()

    with tile.TileContext(nc) as tc:
        with ExitStack() as ctx:
            _emit(ctx, tc, nc, xt, wt, eb, idx_out, w_out, G, repeat)
    nc.compile()
    return nc


def _emit(ctx, tc, nc, xt, wt, eb, idx_out, w_out, G, repeat=1):
    const = ctx.enter_context(tc.tile_pool(name="const", bufs=1))
    xtp = ctx.enter_context(tc.tile_pool(name="xtp", bufs=4))
    psB = ctx.enter_context(tc.tile_pool(name="psB", bufs=3, space="PSUM"))
    wk = ctx.enter_context(tc.tile_pool(name="wk", bufs=3))
    ohp = ctx.enter_context(tc.tile_pool(name="ohp", bufs=3))
    outp = ctx.enter_context(tc.tile_pool(name="outp", bufs=3))

    # Constants: W^T chunks (streamed matmul operand), broadcast biases,
    # expert-id iota row.
    wt_sb = const.tile([128, KH, E], f32)
    nc.sync.dma_start(out=wt_sb,
                      in_=wt.rearrange("(k p) e -> p k e", p=128))
    bias_sb = const.tile([128, E], f32)
    nc.gpsimd.dma_start(out=bias_sb, in_=eb.unsqueeze(0).broadcast_to((128, E)))
    iota = const.tile([128, E], f32)
    nc.gpsimd.iota(iota, pattern=[[1, E]], base=0, channel_multiplier=0,
                   allow_small_or_imprecise_dtypes=True)

    # PE matmuls lower to LDW+MM structs that can carry only ONE sync wait.
    # Consume the W^T DMA dep with a single-wait PE warmup op so loop matmuls
    # each need at most one (their x-tile DMA).
    scr = ctx.enter_context(tc.tile_pool(name="scr", bufs=1, space="PSUM"))
    warm_m = scr.tile([64, 64], f32, tag="warm_m")
    nc.tensor.matmul(warm_m, lhsT=wt_sb[:, 0, :],
                     rhs=wt_sb[:, 0, :], start=True, stop=True)
    # likewise pre-consume the bias-broadcast DMA on DVE and ACT
    warm_v = const.tile([128, 1], f32, tag="warm_v")
    nc.vector.tensor_copy(warm_v, bias_sb[:, 0:1])
    warm_a = const.tile([128, 1], f32, tag="warm_a")
    nc.scalar.copy(warm_a, bias_sb[:, 0:1])
    # Pool instructions aren't ordered across Q7 cores: consume the iota
    # production tick on Pool's own sem too
    warm_p2 = const.tile([128, 1], f32, tag="warm_p2")
    nc.gpsimd.tensor_copy(warm_p2, iota[:, 0:1])

    xt_r = xt.rearrange("(k p) (g t) -> g p k t", p=128, t=TG)

    # gather helpers kept 3D (walrus limits STT/TT inputs to 2-3 dims)
    bias_b3 = bias_sb.unsqueeze(1).to_broadcast([128, NSUB * K, E])
    iota_b3 = iota.unsqueeze(1).to_broadcast([128, NSUB * K, E])

    KQ = 4               # x-tile DMA split: KH/KQ chunks per sub-DMA
    KHQ = KH // KQ

    for g in [g for _ in range(repeat) for g in range(G)]:
        # ---- load x^T group as KQ separate chunk tiles [128h, KHQ, 512t]
        # so PE can start accumulating after the first 1MB lands
        xgs = []
        for q in range(KQ):
            xq = xtp.tile([128, KHQ, TG], f32, tag=f"xg{q}")
            nc.sync.dma_start(out=xq, in_=xt_r[g][:, q * KHQ:(q + 1) * KHQ, :])
            xgs.append(xq)

        # ---- logits [128t, NSUB, 64e]: x^T chunk stationary, W^T streams.
        # Accumulation groups must stay contiguous per PSUM region (HW
        # verified: interleaving corrupts results), so j outer / k inner;
        # j=0 still starts as soon as the first chunk DMA lands.
        pb = psB.tile([128, NSUB, E], f32, tag="pb")
        for j in range(NSUB):
            for k in range(KH):
                nc.tensor.matmul(pb[:, j, :],
                                 lhsT=xgs[k // KHQ][:, k % KHQ,
                                                    j * 128:(j + 1) * 128],
                                 rhs=wt_sb[:, k, :],
                                 start=(k == 0), stop=(k == KH - 1))

        # ---- softmax over experts (free dim): exp + per-subtile sum on ACT
        sr = wk.tile([128, NSUB, E], f32, tag="sr")
        S_ = wk.tile([128, NSUB], f32, tag="S")
        for j in range(NSUB):
            nc.scalar.activation(sr[:, j, :], pb[:, j, :], func=Act.Exp,
                                 accum_out=S_[:, j:j + 1])
        R_ = wk.tile([128, NSUB], f32, tag="R")
        nc.vector.reciprocal(R_, S_)

        # ---- bias-adjusted scores z = exp*R + bias, and top-8
        z_ = wk.tile([128, NSUB, E], f32, tag="z")
        v_ = wk.tile([128, NSUB, K], f32, tag="v")
        ix = wk.tile([128, NSUB, K], u16, tag="ix")
        for j in range(NSUB):
            nc.vector.scalar_tensor_tensor(z_[:, j, :], sr[:, j, :],
                                           R_[:, j:j + 1], bias_sb,
                                           Alu.mult, Alu.add)
            nc.vector.max(out=v_[:, j, :], in_=z_[:, j, :])
            nc.vector.max_index(out=ix[:, j, :], in_max=v_[:, j, :],
                                in_values=z_[:, j, :])

        # ---- gather bias[idx] via one-hot arithmetic: s[idx] = v - bias[idx]
        ixf = wk.tile([128, NSUB * K], f32, tag="ixf")
        nc.vector.tensor_copy(ixf, ix.rearrange("p a b -> p (a b)"))
        oh = ohp.tile([128, NSUB * K, E], f32, tag="oh")
        nc.gpsimd.tensor_tensor(oh, ixf.unsqueeze(2).to_broadcast(
            [128, NSUB * K, E]), iota_b3, Alu.subtract)
        nc.vector.scalar_tensor_tensor(oh, oh, 0.0, bias_b3,
                                       Alu.is_equal, Alu.mult)
        bg = wk.tile([128, NSUB, K], f32, tag="bg")
        nc.vector.tensor_reduce(bg.rearrange("p a b -> p (a b)"), oh,
                                axis=Ax.X, op=Alu.add)
        sg = wk.tile([128, NSUB, K], f32, tag="sg")
        nc.vector.tensor_sub(sg, v_, bg)

        # ---- normalize top-8 weights; emit outputs
        S8 = wk.tile([128, NSUB], f32, tag="S8")
        nc.vector.tensor_reduce(S8, sg, axis=Ax.X, op=Alu.add)
        R8 = wk.tile([128, NSUB], f32, tag="R8")
        nc.vector.reciprocal(R8, S8)
        w_g = outp.tile([128, NSUB, K], f32, tag="w_g")
        for j in range(NSUB):
            nc.scalar.activation(w_g[:, j, :], sg[:, j, :], func=Act.Copy,
                                 scale=R8[:, j:j + 1])
        idx_g = outp.tile([128, NSUB, K], i32, tag="idx_g")
        nc.vector.tensor_copy(idx_g, ix)
        # per-group stores overlap with later groups' compute; issue on the
        # ACT DGE ring so they don't head-of-line-block sync-ring loads
        nc.scalar.dma_start(out=idx_out[:, g], in_=idx_g)
        nc.scalar.dma_start(out=w_out[:, g], in_=w_g)


_NC_CACHE = {}


def get_nc(t_core=T_CORE, repeat=1):
    key = (t_core, repeat)
    if key not in _NC_CACHE:
        _NC_CACHE[key] = build_nc(t_core, repeat)
    return _NC_CACHE[key]


def _reorder(dev_out, t_core):
    # [128, G, NSUB, K] -> [t_core, K] with token = (g*NSUB+j)*128 + p
    return dev_out.transpose(1, 2, 0, 3).reshape(t_core, K)


def kernel(hidden_states, weight, expert_biases, top_k):
    from concourse.bass_utils import run_bass_kernel_spmd

    assert int(top_k) == K
    x2d = np.asarray(hidden_states, dtype=np.float32).reshape(-1, H)
    wt = np.ascontiguousarray(np.asarray(weight, dtype=np.float32).T)
    eb = np.ascontiguousarray(np.asarray(expert_biases, dtype=np.float32))

    nc = get_nc()
    in_maps = []
    for c in range(N_CORES):
        xc = np.ascontiguousarray(x2d[c * T_CORE:(c + 1) * T_CORE, :].T)
        in_maps.append({"xt": xc, "wt": wt, "eb": eb})
    res = run_bass_kernel_spmd(nc, in_maps, core_ids=list(range(N_CORES)))

    idxs, ws = [], []
    for c in range(N_CORES):
        r = res.results[c]
        idxs.append(_reorder(r["idx_out"], T_CORE))
        ws.append(_reorder(r["w_out"], T_CORE))
    return (np.concatenate(idxs, axis=0).astype(np.int32),
            np.concatenate(ws, axis=0).astype(np.float32))



# revision 9
# speedup vs baseline: 1.5455x; 1.5455x over previous
"""MoE gate (softmax + bias-adjusted top-8 routing) Trainium2 Bass kernel.

Full inputs in, full outputs out. Token dim (B*S = 32768) is sharded 8 ways
across NeuronCores; the tiny gate weight [E,H] and expert biases [E] are
replicated.

v4 design:
  - fp16 hi/lo matmul: x is host-split into xh = fp16(x), xl = fp16(x-xh)
    (exact to ~2^-23 together) and W^T chunks are packed [Wh_k | Wl_k]
    [128h, 128] fp16 stationary. Streaming xh then xl into one PSUM
    [128, 512] accumulates top = (xh+xl)@Wh, bottom = (xh+xl)@Wl, so
    top+bottom is the exact fp32-grade logit. fp16 streams at 1 col/cycle
    (~259ns per 512-token chunk MM) vs fp32's 2 passes (~858ns).
  - The half-sum + transpose happen in ONE data-stationary matmul per
    128-token tile: out[t,e] = sum_k psum_copy[k,t]*Sel[k,e] with
    Sel = [I64; I64] (host input), yielding token-major logits [128t,64e].
  - exp + per-subtile softmax denominator fused on ACT (accum_out).
  - Ranking by y = exp + S*b, order-equivalent to softmax+bias (S>0).
    v = max8(y); b[ix] via exact one-hot gather (gpsimd eq-pass, DVE
    mask*bias + reduce). nsg = S*b[ix] - v = -S*softmax_sel; the common
    -S factor cancels in w = nsg / sum(nsg).
  - Software-pipelined emission: PE runs group g's logits while group
    g-1's selector matmuls wait on their ACT copy, removing the
    exp/selector head-of-line stall observed in v3.
"""

import os
import sys
from contextlib import ExitStack

import numpy as np

sys.path.insert(0, "/opt/trn_rl_repo")

import concourse.bacc as bacc
import concourse.bass as bass
import concourse.mybir as mybir
import concourse.tile as tile

B, S, H, E, K = 8, 4096, 2048, 64, 8
N_CORES = 8
T = B * S
T_CORE = T // N_CORES  # 4096 tokens per core
TG = 512               # tokens per group
NSUB = TG // 128       # 128-token subtiles per group
KH = H // 128          # contraction chunks

f32 = mybir.dt.float32
f16 = mybir.dt.float16
i32 = mybir.dt.int32
u32 = mybir.dt.uint32
Alu = mybir.AluOpType
Act = mybir.ActivationFunctionType
Ax = mybir.AxisListType


def build_nc(t_core=T_CORE, repeat=1):
    G = t_core // TG
    nc = bacc.Bacc("TRN2", target_bir_lowering=False, debug=False,
                   enable_asserts=False)
    xh = nc.dram_tensor("xh", [H, t_core], f16, kind="ExternalInput").ap()
    xl = nc.dram_tensor("xl", [H, t_core], f16, kind="ExternalInput").ap()
    whl = nc.dram_tensor("whl", [H, 2 * E], f16, kind="ExternalInput").ap()
    sel = nc.dram_tensor("sel", [128, E], f32, kind="ExternalInput").ap()
    eb = nc.dram_tensor("eb", [E], f32, kind="ExternalInput").ap()
    iot = nc.dram_tensor("iot", [E], f32, kind="ExternalInput").ap()
    # Outputs in [128p, G, NSUB, K] layout (token = (g*NSUB+j)*128 + p) so the
    # store DMA is fully contiguous; host reorders the tiny result.
    idx_out = nc.dram_tensor("idx_out", [128, G, NSUB, K], u32,
                             kind="ExternalOutput").ap()
    w_out = nc.dram_tensor("w_out", [128, G, NSUB, K], f32,
                           kind="ExternalOutput").ap()

    with tile.TileContext(nc) as tc:
        with ExitStack() as ctx:
            _emit(ctx, tc, nc, xh, xl, whl, sel, eb, iot,
                  idx_out, w_out, G, repeat)
    nc.compile()
    return nc


def _emit(ctx, tc, nc, xh, xl, whl, sel, eb, iot, idx_out, w_out, G,
          repeat=1):
    const = ctx.enter_context(tc.tile_pool(name="const", bufs=1))
    xtp = ctx.enter_context(tc.tile_pool(name="xtp", bufs=3))
    psl = ctx.enter_context(tc.tile_pool(name="psl", bufs=2, space="PSUM"))
    pst = ctx.enter_context(tc.tile_pool(name="pst", bufs=2, space="PSUM"))
    wk = ctx.enter_context(tc.tile_pool(name="wk", bufs=2))
    outp = ctx.enter_context(tc.tile_pool(name="outp", bufs=3))

    # Constants: packed [Wh|Wl] chunks (stationary), selector [I;I],
    # broadcast biases, expert-id iota row.
    whl_sb = const.tile([128, KH, 2 * E], f16)
    nc.sync.dma_start(out=whl_sb,
                      in_=whl.rearrange("(k p) e -> p k e", p=128))
    sel_sb = const.tile([128, E], f32)
    nc.sync.dma_start(out=sel_sb, in_=sel)
    bias_sb = const.tile([128, E], f32)
    nc.gpsimd.dma_start(out=bias_sb, in_=eb.unsqueeze(0).broadcast_to((128, E)))
    iota = const.tile([128, E], f32)
    nc.gpsimd.dma_start(out=iota, in_=iot.unsqueeze(0).broadcast_to((128, E)))

    # PE matmuls lower to LDW+MM structs that can carry only ONE sync wait.
    # Consume the W/sel DMA deps with single-wait PE warmup ops so loop
    # matmuls each need at most one (their x-tile DMA).
    scr = ctx.enter_context(tc.tile_pool(name="scr", bufs=1, space="PSUM"))
    warm_m = scr.tile([64, 64], f32, tag="warm_m")
    nc.tensor.matmul(warm_m, lhsT=whl_sb[:, 0, 0:E], rhs=whl_sb[:, 0, 0:E],
                     start=True, stop=True)
    nc.tensor.matmul(warm_m, lhsT=sel_sb[:, 0:64], rhs=sel_sb, start=True,
                     stop=True)
    # pre-consume the bias/iota broadcasts on the engines that read them
    warm_p = const.tile([128, 1], f32, tag="warm_p")
    nc.gpsimd.tensor_copy(warm_p, bias_sb[:, 0:1])
    nc.gpsimd.tensor_copy(warm_p, iota[:, 0:1])
    warm_v = const.tile([128, 1], f32, tag="warm_v")
    nc.vector.tensor_copy(warm_v, bias_sb[:, 0:1])
    nc.vector.tensor_copy(warm_v, iota[:, 0:1])
    warm_a = const.tile([128, 1], f32, tag="warm_a")
    nc.scalar.copy(warm_a, bias_sb[:, 0:1])

    xh_r = xh.rearrange("(k p) (g t) -> g p k t", p=128, t=TG)
    xl_r = xl.rearrange("(k p) (g t) -> g p k t", p=128, t=TG)

    # gather helpers kept 3D (walrus limits STT/TT inputs to 2-3 dims)
    bias_b3 = bias_sb.unsqueeze(1).to_broadcast([128, NSUB * K, E])
    iota_b3 = iota.unsqueeze(1).to_broadcast([128, NSUB * K, E])

    KQ = 2               # x-tile DMA split: KH/KQ chunks per sub-DMA
    KHQ = KH // KQ

    state = {}

    def stage_a(g):
        # ---- load the group's xh then xl chunk tiles; PE streams hi first
        xgs_h, xgs_l = [], []
        for q in range(KQ):
            xq = xtp.tile([128, KHQ, TG], f16, tag=f"xh{q}")
            nc.sync.dma_start(out=xq, in_=xh_r[g][:, q * KHQ:(q + 1) * KHQ, :])
            xgs_h.append(xq)
        # hi stream rides the SP ring, lo stream the ACT ring: the two
        # hardware DGE rings process transfers serially per-ring, so
        # splitting the streams doubles effective load bandwidth
        for q in range(KQ):
            xq = xtp.tile([128, KHQ, TG], f16, tag=f"xl{q}")
            nc.scalar.dma_start(out=xq, in_=xl_r[g][:, q * KHQ:(q + 1) * KHQ, :])
            xgs_l.append(xq)

        # ---- [x@Wh ; x@Wl] halves accumulate in one PSUM [128, 512]
        ps_c = psl.tile([128, TG], f32, tag="ps_c")
        for k in range(KH):
            nc.tensor.matmul(ps_c, lhsT=whl_sb[:, k, :],
                             rhs=xgs_h[k // KHQ][:, k % KHQ, :],
                             start=(k == 0), stop=False)
        for k in range(KH):
            nc.tensor.matmul(ps_c, lhsT=whl_sb[:, k, :],
                             rhs=xgs_l[k // KHQ][:, k % KHQ, :],
                             start=False, stop=(k == KH - 1))
        state[g] = ps_c

    def stage_b(g):
        ps_c = state.pop(g)
        # ---- copy halves out of PSUM; selector matmul fuses top+bottom sum
        # with the transpose to token-major logits [128t, 64e]
        pc = wk.tile([128, TG], f32, tag="pc")
        nc.scalar.copy(pc, ps_c)
        ps_t = pst.tile([128, NSUB, E], f32, tag="ps_t")
        for j in range(NSUB):
            nc.tensor.matmul(ps_t[:, j, :],
                             lhsT=pc[:, j * 128:(j + 1) * 128],
                             rhs=sel_sb, start=True, stop=True)
        # ---- exp + fused per-subtile softmax denominator S
        sr = wk.tile([128, NSUB, E], f32, tag="sr")
        S_ = wk.tile([128, NSUB], f32, tag="S")
        for j in range(NSUB):
            nc.scalar.activation(sr[:, j, :], ps_t[:, j, :], func=Act.Exp,
                                 accum_out=S_[:, j:j + 1])

        # ---- rank by y = exp + S*b (== S * (softmax + bias), same order)
        y_ = wk.tile([128, NSUB, E], f32, tag="y")
        v_ = wk.tile([128, NSUB, K], f32, tag="v")
        idx_g = outp.tile([128, NSUB, K], u32, tag="idx_g")
        for j in range(NSUB):
            nc.vector.scalar_tensor_tensor(y_[:, j, :], bias_sb,
                                           S_[:, j:j + 1], sr[:, j, :],
                                           Alu.mult, Alu.add)
            nc.vector.max(out=v_[:, j, :], in_=y_[:, j, :])
            nc.vector.max_index(out=idx_g[:, j, :], in_max=v_[:, j, :],
                                in_values=y_[:, j, :])

        # ---- exact bias gather via one-hot: bg = b[ix]
        ixf = wk.tile([128, NSUB * K], f32, tag="ixf")
        nc.vector.tensor_copy(ixf, idx_g.rearrange("p a b -> p (a b)"))
        oh = wk.tile([128, NSUB * K, E], f32, tag="oh")
        nc.gpsimd.tensor_tensor(oh, ixf.unsqueeze(2).to_broadcast(
            [128, NSUB * K, E]), iota_b3, Alu.subtract)
        nc.vector.scalar_tensor_tensor(oh, oh, 0.0, bias_b3,
                                       Alu.is_equal, Alu.mult)
        bg = wk.tile([128, NSUB, K], f32, tag="bg")
        nc.vector.tensor_reduce(bg.rearrange("p a b -> p (a b)"), oh,
                                axis=Ax.X, op=Alu.add)

        # ---- nsg = S*b[ix] - v = -(exp at winners); the common -S factor
        # cancels in the normalize, so no negation needed
        nsg = wk.tile([128, NSUB, K], f32, tag="nsg")
        for j in range(NSUB):
            nc.vector.scalar_tensor_tensor(nsg[:, j, :], bg[:, j, :],
                                           S_[:, j:j + 1], v_[:, j, :],
                                           Alu.mult, Alu.subtract)
        S8 = wk.tile([128, NSUB], f32, tag="S8")
        nc.vector.tensor_reduce(S8, nsg, axis=Ax.X, op=Alu.add)
        R8 = wk.tile([128, NSUB], f32, tag="R8")
        nc.vector.reciprocal(R8, S8)
        w_g = outp.tile([128, NSUB, K], f32, tag="w_g")
        for j in range(NSUB):
            nc.scalar.activation(w_g[:, j, :], nsg[:, j, :], func=Act.Copy,
                                 scale=R8[:, j:j + 1])
        # per-group stores overlap with later groups' compute; issue on the
        # ACT DGE ring so they don't head-of-line-block sync-ring loads
        nc.scalar.dma_start(out=idx_out[:, g], in_=idx_g)
        nc.scalar.dma_start(out=w_out[:, g], in_=w_g)

    # software-pipelined emission: stage_b(g-1) lands between the logits
    # matmul runs of g and g+1, so PE never waits on ACT mid-stream
    order = [g for _ in range(repeat) for g in range(G)]
    for i, g in enumerate(order):
        stage_a(g)
        if i > 0:
            stage_b(order[i - 1])
    stage_b(order[-1])


_NC_CACHE = {}


def get_nc(t_core=T_CORE, repeat=1):
    key = (t_core, repeat)
    if key not in _NC_CACHE:
        _NC_CACHE[key] = build_nc(t_core, repeat)
    return _NC_CACHE[key]


def _reorder(dev_out, t_core):
    # [128, G, NSUB, K] -> [t_core, K] with token = (g*NSUB+j)*128 + p
    return dev_out.transpose(1, 2, 0, 3).reshape(t_core, K)


def kernel(hidden_states, weight, expert_biases, top_k):
    from concourse.bass_utils import run_bass_kernel_spmd

    assert int(top_k) == K
    x2d = np.asarray(hidden_states, dtype=np.float32).reshape(-1, H)
    w32 = np.asarray(weight, dtype=np.float32).T          # [H, E]
    wh = w32.astype(np.float16)
    wl = (w32 - wh.astype(np.float32)).astype(np.float16)
    whl = np.ascontiguousarray(np.concatenate([wh, wl], axis=1))  # [H, 2E]
    selm = np.ascontiguousarray(
        np.vstack([np.eye(E, dtype=np.float32)] * 2))     # [128, E]
    eb = np.ascontiguousarray(np.asarray(expert_biases, dtype=np.float32))
    iot = np.arange(E, dtype=np.float32)

    nc = get_nc()
    in_maps = []
    for c in range(N_CORES):
        xc = x2d[c * T_CORE:(c + 1) * T_CORE, :].T        # [H, T_CORE] view
        xch = np.ascontiguousarray(xc, dtype=np.float16)
        xcl = np.ascontiguousarray(xc - xch.astype(np.float32),
                                   dtype=np.float16)
        in_maps.append({"xh": xch, "xl": xcl, "whl": whl, "sel": selm,
                        "eb": eb, "iot": iot})
    res = run_bass_kernel_spmd(nc, in_maps, core_ids=list(range(N_CORES)))

    idxs, ws = [], []
    for c in range(N_CORES):
        r = res.results[c]
        idxs.append(_reorder(r["idx_out"], T_CORE))
        ws.append(_reorder(r["w_out"], T_CORE))
    return (np.concatenate(idxs, axis=0).astype(np.int32),
            np.concatenate(ws, axis=0).astype(np.float32))


# revision 10
# speedup vs baseline: 1.7533x; 1.1345x over previous
"""MoE gate (softmax + bias-adjusted top-8 routing) Trainium2 Bass kernel.

Full inputs in, full outputs out. Token dim (B*S = 32768) is sharded 8 ways
across NeuronCores; the tiny gate weight [E,H] and expert biases [E] are
replicated.

v5 design:
  - fp16 hi/lo matmul: x is host-split into xh = fp16(x), xl = fp16(x-xh)
    (exact to ~2^-23 together) and W^T chunks are packed [Wh_k | Wl_k]
    [128h, 128] fp16 stationary. Streaming xh then xl into one PSUM
    [128, 512] accumulates top = (xh+xl)@Wh, bottom = (xh+xl)@Wl, so
    top+bottom is the exact fp32-grade logit. fp16 streams at 1 col/cycle
    (~220ns per 512-token chunk MM) vs fp32's 2 passes (~858ns).
  - The half-sum + transpose happen in ONE data-stationary matmul per
    128-token tile: out[t,e] = sum_k psum_copy[k,t]*Sel[k,e] with
    Sel = [I64; I64] (host input), yielding token-major logits [128t,64e].
  - exp + per-subtile softmax denominator fused on ACT (accum_out).
  - Ranking by y = exp + S*b, order-equivalent to softmax+bias (S>0).
    The device emits idx = top-8 indices, v = max8(y) values, and S;
    the tiny host epilogue recovers the winners' softmax probs
    p = v/S - b[idx] and normalizes (exact; no device-side gather).
  - Software-pipelined emission: PE runs group g's logits while group
    g-1's selector matmuls wait on their ACT copy. x loads split across
    the SP hardware DGE ring (hi) and the gpsimd software ring (lo) so
    no single ring serializes the 4MB/group input stream; the ACT ring
    carries only the small result stores.
"""

import os
import sys
from contextlib import ExitStack

import numpy as np

sys.path.insert(0, "/opt/trn_rl_repo")

import concourse.bacc as bacc
import concourse.bass as bass
import concourse.mybir as mybir
import concourse.tile as tile

B, S, H, E, K = 8, 4096, 2048, 64, 8
N_CORES = 8
T = B * S
T_CORE = T // N_CORES  # 4096 tokens per core
TG = 512               # tokens per group
NSUB = TG // 128       # 128-token subtiles per group
KH = H // 128          # contraction chunks

f32 = mybir.dt.float32
f16 = mybir.dt.float16
i32 = mybir.dt.int32
u32 = mybir.dt.uint32
Alu = mybir.AluOpType
Act = mybir.ActivationFunctionType
Ax = mybir.AxisListType


def build_nc(t_core=T_CORE, repeat=1):
    G = t_core // TG
    nc = bacc.Bacc("TRN2", target_bir_lowering=False, debug=False,
                   enable_asserts=False)
    xh = nc.dram_tensor("xh", [H, t_core], f16, kind="ExternalInput").ap()
    xl = nc.dram_tensor("xl", [H, t_core], f16, kind="ExternalInput").ap()
    whl = nc.dram_tensor("whl", [H, 2 * E], f16, kind="ExternalInput").ap()
    sel = nc.dram_tensor("sel", [128, E], f32, kind="ExternalInput").ap()
    eb = nc.dram_tensor("eb", [E], f32, kind="ExternalInput").ap()
    # Outputs in [128p, G, ...] layout (token = (g*NSUB+j)*128 + p) so the
    # store DMAs are fully contiguous; host reorders the tiny results.
    idx_out = nc.dram_tensor("idx_out", [128, G, NSUB, K], u32,
                             kind="ExternalOutput").ap()
    v_out = nc.dram_tensor("v_out", [128, G, NSUB, K], f32,
                           kind="ExternalOutput").ap()
    s_out = nc.dram_tensor("s_out", [128, G, NSUB], f32,
                           kind="ExternalOutput").ap()

    with tile.TileContext(nc) as tc:
        with ExitStack() as ctx:
            _emit(ctx, tc, nc, xh, xl, whl, sel, eb,
                  idx_out, v_out, s_out, G, repeat)
    nc.compile()
    return nc


def _emit(ctx, tc, nc, xh, xl, whl, sel, eb, idx_out, v_out, s_out, G,
          repeat=1):
    const = ctx.enter_context(tc.tile_pool(name="const", bufs=1))
    xtp = ctx.enter_context(tc.tile_pool(name="xtp", bufs=3))
    psl = ctx.enter_context(tc.tile_pool(name="psl", bufs=2, space="PSUM"))
    pst = ctx.enter_context(tc.tile_pool(name="pst", bufs=2, space="PSUM"))
    wk = ctx.enter_context(tc.tile_pool(name="wk", bufs=2))
    outp = ctx.enter_context(tc.tile_pool(name="outp", bufs=3))

    # Constants: packed [Wh|Wl] chunks (stationary), selector [I;I],
    # broadcast biases.
    whl_sb = const.tile([128, KH, 2 * E], f16)
    nc.sync.dma_start(out=whl_sb,
                      in_=whl.rearrange("(k p) e -> p k e", p=128))
    sel_sb = const.tile([128, E], f32)
    nc.sync.dma_start(out=sel_sb, in_=sel)
    bias_sb = const.tile([128, E], f32)
    nc.gpsimd.dma_start(out=bias_sb, in_=eb.unsqueeze(0).broadcast_to((128, E)))

    # PE matmuls lower to LDW+MM structs that can carry only ONE sync wait.
    # Consume the W/sel DMA deps with single-wait PE warmup ops so loop
    # matmuls each need at most one (their x-tile DMA).
    scr = ctx.enter_context(tc.tile_pool(name="scr", bufs=1, space="PSUM"))
    warm_m = scr.tile([64, 64], f32, tag="warm_m")
    nc.tensor.matmul(warm_m, lhsT=whl_sb[:, 0, 0:E], rhs=whl_sb[:, 0, 0:E],
                     start=True, stop=True)
    nc.tensor.matmul(warm_m, lhsT=sel_sb[:, 0:64], rhs=sel_sb, start=True,
                     stop=True)
    # pre-consume the bias broadcast on the engine that reads it (DVE)
    warm_v = const.tile([128, 1], f32, tag="warm_v")
    nc.vector.tensor_copy(warm_v, bias_sb[:, 0:1])

    xh_r = xh.rearrange("(k p) (g t) -> g p k t", p=128, t=TG)
    xl_r = xl.rearrange("(k p) (g t) -> g p k t", p=128, t=TG)

    KQ = 2               # x-tile DMA split: KH/KQ chunks per sub-DMA
    KHQ = KH // KQ

    state = {}

    def stage_a(g):
        # ---- load the group's xh then xl chunk tiles; PE streams hi first.
        # hi rides the SP hardware ring, lo the gpsimd software ring, so the
        # two 2MB streams transfer concurrently.
        xgs_h, xgs_l = [], []
        for q in range(KQ):
            xq = xtp.tile([128, KHQ, TG], f16, tag=f"xh{q}")
            nc.sync.dma_start(out=xq, in_=xh_r[g][:, q * KHQ:(q + 1) * KHQ, :])
            xgs_h.append(xq)
        for q in range(KQ):
            xq = xtp.tile([128, KHQ, TG], f16, tag=f"xl{q}")
            nc.gpsimd.dma_start(out=xq,
                                in_=xl_r[g][:, q * KHQ:(q + 1) * KHQ, :])
            xgs_l.append(xq)

        # ---- [x@Wh ; x@Wl] halves accumulate in one PSUM [128, 512]
        ps_c = psl.tile([128, TG], f32, tag="ps_c")
        for k in range(KH):
            nc.tensor.matmul(ps_c, lhsT=whl_sb[:, k, :],
                             rhs=xgs_h[k // KHQ][:, k % KHQ, :],
                             start=(k == 0), stop=False)
        for k in range(KH):
            nc.tensor.matmul(ps_c, lhsT=whl_sb[:, k, :],
                             rhs=xgs_l[k // KHQ][:, k % KHQ, :],
                             start=False, stop=(k == KH - 1))
        state[g] = ps_c

    def stage_b(g):
        ps_c = state.pop(g)
        # ---- copy halves out of PSUM; selector matmul fuses top+bottom sum
        # with the transpose to token-major logits [128t, 64e]
        pc = wk.tile([128, TG], f32, tag="pc")
        nc.scalar.copy(pc, ps_c)
        ps_t = pst.tile([128, NSUB, E], f32, tag="ps_t")
        for j in range(NSUB):
            nc.tensor.matmul(ps_t[:, j, :],
                             lhsT=pc[:, j * 128:(j + 1) * 128],
                             rhs=sel_sb, start=True, stop=True)
        # ---- exp + fused per-subtile softmax denominator S
        sr = wk.tile([128, NSUB, E], f32, tag="sr")
        S_ = outp.tile([128, NSUB], f32, tag="S")
        for j in range(NSUB):
            nc.scalar.activation(sr[:, j, :], ps_t[:, j, :], func=Act.Exp,
                                 accum_out=S_[:, j:j + 1])

        # ---- rank by y = exp + S*b (== S * (softmax + bias), same order)
        y_ = wk.tile([128, NSUB, E], f32, tag="y")
        v_ = outp.tile([128, NSUB, K], f32, tag="v")
        idx_g = outp.tile([128, NSUB, K], u32, tag="idx_g")
        for j in range(NSUB):
            nc.vector.scalar_tensor_tensor(y_[:, j, :], bias_sb,
                                           S_[:, j:j + 1], sr[:, j, :],
                                           Alu.mult, Alu.add)
            nc.vector.max(out=v_[:, j, :], in_=y_[:, j, :])
            nc.vector.max_index(out=idx_g[:, j, :], in_max=v_[:, j, :],
                                in_values=y_[:, j, :])

        # per-group stores overlap with later groups' compute; the ACT DGE
        # ring carries only these small results
        nc.scalar.dma_start(out=idx_out[:, g], in_=idx_g)
        nc.scalar.dma_start(out=v_out[:, g], in_=v_)
        nc.scalar.dma_start(out=s_out[:, g], in_=S_)

    # software-pipelined emission: stage_b(g-1) lands between the logits
    # matmul runs of g and g+1, so PE never waits on ACT mid-stream
    order = [g for _ in range(repeat) for g in range(G)]
    for i, g in enumerate(order):
        stage_a(g)
        if i > 0:
            stage_b(order[i - 1])
    stage_b(order[-1])


_NC_CACHE = {}


def get_nc(t_core=T_CORE, repeat=1):
    key = (t_core, repeat)
    if key not in _NC_CACHE:
        _NC_CACHE[key] = build_nc(t_core, repeat)
    return _NC_CACHE[key]


def _reorder(dev_out, t_core):
    # [128, G, NSUB, ...] -> [t_core, ...] with token = (g*NSUB+j)*128 + p
    d = dev_out
    rest = d.shape[3:]
    return d.transpose(1, 2, 0, 3).reshape((t_core,) + rest) if rest else \
        d.transpose(1, 2, 0).reshape(t_core)


def kernel(hidden_states, weight, expert_biases, top_k):
    from concourse.bass_utils import run_bass_kernel_spmd

    assert int(top_k) == K
    x2d = np.asarray(hidden_states, dtype=np.float32).reshape(-1, H)
    w32 = np.asarray(weight, dtype=np.float32).T          # [H, E]
    wh = w32.astype(np.float16)
    wl = (w32 - wh.astype(np.float32)).astype(np.float16)
    whl = np.ascontiguousarray(np.concatenate([wh, wl], axis=1))  # [H, 2E]
    selm = np.ascontiguousarray(
        np.vstack([np.eye(E, dtype=np.float32)] * 2))     # [128, E]
    eb = np.ascontiguousarray(np.asarray(expert_biases, dtype=np.float32))

    nc = get_nc()
    in_maps = []
    for c in range(N_CORES):
        xc = x2d[c * T_CORE:(c + 1) * T_CORE, :].T        # [H, T_CORE] view
        xch = np.ascontiguousarray(xc, dtype=np.float16)
        xcl = np.ascontiguousarray(xc - xch.astype(np.float32),
                                   dtype=np.float16)
        in_maps.append({"xh": xch, "xl": xcl, "whl": whl, "sel": selm,
                        "eb": eb})
    res = run_bass_kernel_spmd(nc, in_maps, core_ids=list(range(N_CORES)))

    idxs, vs, ss = [], [], []
    for c in range(N_CORES):
        r = res.results[c]
        idxs.append(_reorder(r["idx_out"], T_CORE))
        vs.append(_reorder(r["v_out"], T_CORE))
        ss.append(_reorder(r["s_out"], T_CORE).reshape(T_CORE))
    idx = np.concatenate(idxs, axis=0)                    # [T, K] u32
    v = np.concatenate(vs, axis=0).astype(np.float64)     # [T, K]
    s = np.concatenate(ss, axis=0).astype(np.float64)     # [T]
    # host epilogue: winners' softmax probs p = v/S - b[idx], normalized
    p = v / s[:, None] - eb.astype(np.float64)[idx]
    w = p / (p.sum(axis=1, keepdims=True) + 1e-20)
    return idx.astype(np.int32), w.astype(np.float32)


# revision 11
# speedup vs baseline: 1.9864x; 1.1330x over previous
"""MoE gate (softmax + bias-adjusted top-8 routing) Trainium2 Bass kernel.

Full inputs in, full outputs out. Token dim (B*S = 32768) is sharded 8 ways
across NeuronCores; the tiny gate weight [E,H] and expert biases [E] are
replicated.

v5 design:
  - fp16 hi/lo matmul: x is host-split into xh = fp16(x), xl = fp16(x-xh)
    (exact to ~2^-23 together) and W^T chunks are packed [Wh_k | Wl_k]
    [128h, 128] fp16 stationary. Streaming xh then xl into one PSUM
    [128, 512] accumulates top = (xh+xl)@Wh, bottom = (xh+xl)@Wl, so
    top+bottom is the exact fp32-grade logit. fp16 streams at 1 col/cycle
    (~220ns per 512-token chunk MM) vs fp32's 2 passes (~858ns).
  - The half-sum + transpose happen in ONE data-stationary matmul per
    128-token tile: out[t,e] = sum_k psum_copy[k,t]*Sel[k,e] with
    Sel = [I64; I64] (host input), yielding token-major logits [128t,64e].
  - exp + per-subtile softmax denominator fused on ACT (accum_out).
  - Ranking by y = exp + S*b, order-equivalent to softmax+bias (S>0).
    The device emits idx = top-8 indices, v = max8(y) values, and S;
    the tiny host epilogue recovers the winners' softmax probs
    p = v/S - b[idx] and normalizes (exact; no device-side gather).
  - Software-pipelined emission: PE runs group g's logits while group
    g-1's selector matmuls wait on their ACT copy. x loads split across
    the SP hardware DGE ring (hi) and the gpsimd software ring (lo) so
    no single ring serializes the 4MB/group input stream; the ACT ring
    carries only the small result stores.
"""

import os
import sys
from contextlib import ExitStack

import numpy as np

sys.path.insert(0, "/opt/trn_rl_repo")

import concourse.bacc as bacc
import concourse.bass as bass
import concourse.mybir as mybir
import concourse.tile as tile

B, S, H, E, K = 8, 4096, 2048, 64, 8
N_CORES = 8
T = B * S
T_CORE = T // N_CORES  # 4096 tokens per core
TG = 512               # tokens per group
NSUB = TG // 128       # 128-token subtiles per group
KH = H // 128          # contraction chunks

f32 = mybir.dt.float32
f16 = mybir.dt.float16
i32 = mybir.dt.int32
u32 = mybir.dt.uint32
Alu = mybir.AluOpType
Act = mybir.ActivationFunctionType
Ax = mybir.AxisListType


def build_nc(t_core=T_CORE, repeat=1):
    G = t_core // TG
    nc = bacc.Bacc("TRN2", target_bir_lowering=False, debug=False,
                   enable_asserts=False)
    xi = nc.dram_tensor("xi", [H, 2 * t_core], f16, kind="ExternalInput").ap()
    whl = nc.dram_tensor("whl", [H, 2 * E], f16, kind="ExternalInput").ap()
    sel = nc.dram_tensor("sel", [128, E], f32, kind="ExternalInput").ap()
    eb = nc.dram_tensor("eb", [E], f32, kind="ExternalInput").ap()
    # Outputs in [128p, G, ...] layout (token = (g*NSUB+j)*128 + p) so the
    # store DMAs are fully contiguous; host reorders the tiny results.
    idx_out = nc.dram_tensor("idx_out", [128, G, NSUB, K], u32,
                             kind="ExternalOutput").ap()
    v_out = nc.dram_tensor("v_out", [128, G, NSUB, K], f32,
                           kind="ExternalOutput").ap()
    s_out = nc.dram_tensor("s_out", [128, G, NSUB], f32,
                           kind="ExternalOutput").ap()

    with tile.TileContext(nc) as tc:
        with ExitStack() as ctx:
            _emit(ctx, tc, nc, xi, whl, sel, eb,
                  idx_out, v_out, s_out, G, repeat)
    nc.compile()
    return nc


def _emit(ctx, tc, nc, xi, whl, sel, eb, idx_out, v_out, s_out, G,
          repeat=1):
    const = ctx.enter_context(tc.tile_pool(name="const", bufs=1))
    xtp = ctx.enter_context(tc.tile_pool(name="xtp", bufs=3))
    psl = ctx.enter_context(tc.tile_pool(name="psl", bufs=2, space="PSUM"))
    pst = ctx.enter_context(tc.tile_pool(name="pst", bufs=2, space="PSUM"))
    wk = ctx.enter_context(tc.tile_pool(name="wk", bufs=2))
    outp = ctx.enter_context(tc.tile_pool(name="outp", bufs=3))

    # Constants: packed [Wh|Wl] chunks (stationary), selector [I;I],
    # broadcast biases.
    whl_sb = const.tile([128, KH, 2 * E], f16)
    nc.sync.dma_start(out=whl_sb,
                      in_=whl.rearrange("(k p) e -> p k e", p=128))
    sel_sb = const.tile([128, E], f32)
    nc.sync.dma_start(out=sel_sb, in_=sel)
    bias_sb = const.tile([128, E], f32)
    nc.gpsimd.dma_start(out=bias_sb, in_=eb.unsqueeze(0).broadcast_to((128, E)))

    # PE matmuls lower to LDW+MM structs that can carry only ONE sync wait.
    # Consume the W/sel DMA deps with single-wait PE warmup ops so loop
    # matmuls each need at most one (their x-tile DMA).
    scr = ctx.enter_context(tc.tile_pool(name="scr", bufs=1, space="PSUM"))
    warm_m = scr.tile([64, 64], f32, tag="warm_m")
    nc.tensor.matmul(warm_m, lhsT=whl_sb[:, 0, 0:E], rhs=whl_sb[:, 0, 0:E],
                     start=True, stop=True)
    nc.tensor.matmul(warm_m, lhsT=sel_sb[:, 0:64], rhs=sel_sb, start=True,
                     stop=True)
    # pre-consume the bias broadcast on the engine that reads it (DVE)
    warm_v = const.tile([128, 1], f32, tag="warm_v")
    nc.vector.tensor_copy(warm_v, bias_sb[:, 0:1])

    # x ships as [hi 512-token block | lo 512-token block] per group so
    # every DMA partition line is 2KB (1KB f16 lines halve DMA efficiency)
    xi_r = xi.rearrange("(k p) (g u) -> g p k u", p=128, u=2 * TG)

    KQ = 2               # x-tile DMA split: KH/KQ chunks per sub-DMA
    KHQ = KH // KQ

    state = {}

    def stage_a(g):
        # ---- load the group's [hi|lo] chunk tiles; the first chunk half
        # rides the SP hardware ring, the second the gpsimd software ring,
        # so the two 2MB halves transfer concurrently.
        xgs = []
        xq = xtp.tile([128, KHQ, 2 * TG], f16, tag="xa")
        nc.sync.dma_start(out=xq, in_=xi_r[g][:, 0:KHQ, :])
        xgs.append(xq)
        xq = xtp.tile([128, KHQ, 2 * TG], f16, tag="xb")
        nc.gpsimd.dma_start(out=xq, in_=xi_r[g][:, KHQ:KH, :])
        xgs.append(xq)

        # ---- [x@Wh ; x@Wl] halves accumulate in one PSUM [128, 512]
        ps_c = psl.tile([128, TG], f32, tag="ps_c")
        for k in range(KH):
            nc.tensor.matmul(ps_c, lhsT=whl_sb[:, k, :],
                             rhs=xgs[k // KHQ][:, k % KHQ, 0:TG],
                             start=(k == 0), stop=False)
        for k in range(KH):
            nc.tensor.matmul(ps_c, lhsT=whl_sb[:, k, :],
                             rhs=xgs[k // KHQ][:, k % KHQ, TG:2 * TG],
                             start=False, stop=(k == KH - 1))
        state[g] = ps_c

    def stage_b(g):
        ps_c = state.pop(g)
        # ---- copy halves out of PSUM; selector matmul fuses top+bottom sum
        # with the transpose to token-major logits [128t, 64e]
        pc = wk.tile([128, TG], f32, tag="pc")
        nc.scalar.copy(pc, ps_c)
        ps_t = pst.tile([128, NSUB, E], f32, tag="ps_t")
        for j in range(NSUB):
            nc.tensor.matmul(ps_t[:, j, :],
                             lhsT=pc[:, j * 128:(j + 1) * 128],
                             rhs=sel_sb, start=True, stop=True)
        # ---- exp + fused per-subtile softmax denominator S
        sr = wk.tile([128, NSUB, E], f32, tag="sr")
        S_ = outp.tile([128, NSUB], f32, tag="S")
        for j in range(NSUB):
            nc.scalar.activation(sr[:, j, :], ps_t[:, j, :], func=Act.Exp,
                                 accum_out=S_[:, j:j + 1])

        # ---- rank by y = exp + S*b (== S * (softmax + bias), same order)
        y_ = wk.tile([128, NSUB, E], f32, tag="y")
        v_ = outp.tile([128, NSUB, K], f32, tag="v")
        idx_g = outp.tile([128, NSUB, K], u32, tag="idx_g")
        for j in range(NSUB):
            nc.vector.scalar_tensor_tensor(y_[:, j, :], bias_sb,
                                           S_[:, j:j + 1], sr[:, j, :],
                                           Alu.mult, Alu.add)
            nc.vector.max(out=v_[:, j, :], in_=y_[:, j, :])
            nc.vector.max_index(out=idx_g[:, j, :], in_max=v_[:, j, :],
                                in_values=y_[:, j, :])

        # per-group stores overlap with later groups' compute; the ACT DGE
        # ring carries only these small results
        nc.scalar.dma_start(out=idx_out[:, g], in_=idx_g)
        nc.scalar.dma_start(out=v_out[:, g], in_=v_)
        nc.scalar.dma_start(out=s_out[:, g], in_=S_)

    # software-pipelined emission: stage_b(g-1) lands between the logits
    # matmul runs of g and g+1, so PE never waits on ACT mid-stream
    order = [g for _ in range(repeat) for g in range(G)]
    for i, g in enumerate(order):
        stage_a(g)
        if i > 0:
            stage_b(order[i - 1])
    stage_b(order[-1])


_NC_CACHE = {}


def get_nc(t_core=T_CORE, repeat=1):
    key = (t_core, repeat)
    if key not in _NC_CACHE:
        _NC_CACHE[key] = build_nc(t_core, repeat)
    return _NC_CACHE[key]


def _reorder(dev_out, t_core):
    # [128, G, NSUB, ...] -> [t_core, ...] with token = (g*NSUB+j)*128 + p
    d = dev_out
    rest = d.shape[3:]
    return d.transpose(1, 2, 0, 3).reshape((t_core,) + rest) if rest else \
        d.transpose(1, 2, 0).reshape(t_core)


def kernel(hidden_states, weight, expert_biases, top_k):
    from concourse.bass_utils import run_bass_kernel_spmd

    assert int(top_k) == K
    x2d = np.asarray(hidden_states, dtype=np.float32).reshape(-1, H)
    w32 = np.asarray(weight, dtype=np.float32).T          # [H, E]
    wh = w32.astype(np.float16)
    wl = (w32 - wh.astype(np.float32)).astype(np.float16)
    whl = np.ascontiguousarray(np.concatenate([wh, wl], axis=1))  # [H, 2E]
    selm = np.ascontiguousarray(
        np.vstack([np.eye(E, dtype=np.float32)] * 2))     # [128, E]
    eb = np.ascontiguousarray(np.asarray(expert_biases, dtype=np.float32))

    nc = get_nc()
    Gc = T_CORE // TG
    in_maps = []
    for c in range(N_CORES):
        xc = x2d[c * T_CORE:(c + 1) * T_CORE, :].T        # [H, T_CORE] view
        xch = np.asarray(xc, dtype=np.float16)
        xcl = (xc - xch.astype(np.float32)).astype(np.float16)
        xi = np.stack([xch.reshape(H, Gc, TG), xcl.reshape(H, Gc, TG)],
                      axis=2).reshape(H, 2 * T_CORE)
        in_maps.append({"xi": np.ascontiguousarray(xi), "whl": whl,
                        "sel": selm, "eb": eb})
    res = run_bass_kernel_spmd(nc, in_maps, core_ids=list(range(N_CORES)))

    idxs, vs, ss = [], [], []
    for c in range(N_CORES):
        r = res.results[c]
        idxs.append(_reorder(r["idx_out"], T_CORE))
        vs.append(_reorder(r["v_out"], T_CORE))
        ss.append(_reorder(r["s_out"], T_CORE).reshape(T_CORE))
    idx = np.concatenate(idxs, axis=0)                    # [T, K] u32
    v = np.concatenate(vs, axis=0).astype(np.float64)     # [T, K]
    s = np.concatenate(ss, axis=0).astype(np.float64)     # [T]
    # host epilogue: winners' softmax probs p = v/S - b[idx], normalized
    p = v / s[:, None] - eb.astype(np.float64)[idx]
    w = p / (p.sum(axis=1, keepdims=True) + 1e-20)
    return idx.astype(np.int32), w.astype(np.float32)
